# revision 11
# baseline (speedup 1.0000x reference)
"""Trainium2 Bass kernel: DeformableValueAttention.

Full-input contract: kernel(**inputs) takes the unsharded inputs of
reference.setup_inputs() and returns the full [B, C, H, W] output.

Sharding: 8 cores = (batch b, query-half qh). Each core computes attention
for 512 queries x all 1024 keys of one batch and produces a disjoint
[C, 512] column-slice of the output -- no cross-core reduction.

Per-core algorithm (channels-on-partitions layouts):
  QT = (Wq/8) @ xq          [C, Nq]
  KT = Wk @ xkv             [C, N]
  V  = xkv^T @ Wv^T         [N, C]    (keys on partitions)
  Vd = G^T.T @ V            [N, C]    grid_sample as banded sparse matmul,
                                      (1+gamma*sal) folded into G
  per head pair t (2-head PE row-group packing, K=64):
    S^T = KT_t^T @ QT_t     [N, 2*Nq] scores, both heads side by side
    Pu  = exp(S^T)          bf16      (no max-subtraction: |scores| < ~6)
    O   = [Vd_h | 1]^T @ Pu [65, Nq]  per head; ones-column gives the
                                      softmax denominator in row 64
  normalize: DVE reciprocal of denom row + GPSIMD partition_broadcast +
             DVE multiply (no ACT -> no activation-table ping-pong);
             normalized heads land pairwise in [128, Nq] channel tiles
  out^T = Wo @ O            [C, Nq]   K=128 GEMM over the 4 pair tiles

Schedule notes (static in-order engine queues -> emission order is the
schedule):
  - Inputs arrive via one fat DMA per tensor, spread over the 4 engine
    queues in criticality order so the first exp fires ~6us earlier.
  - The 32 exp activations are the ACT pacer (~1.1us each); V, Vd and the
    first half of the O matmuls are interleaved into the S stream so the
    PE stays busy under the exp stream.
  - PSUM: two [128,1024] tags x 2 bufs = all 8 banks. O accumulates head A
    in bank cols 0:512 and head B in 512:1024 of one tile.
  - Output DMA is split per 128-row tile across all 4 queues.

Fidelity notes:
  - P_thermal adds a per-query constant pre-softmax; softmax is invariant.
  - Zero biases assumed (asserted); numpy fallback otherwise.
  - bf16 matmuls, fp32 PSUM; measured rel err ~5e-3 vs fp32 reference.
"""

import os
import sys

import numpy as np
import ml_dtypes

try:
    import concourse.bass as bass
except ImportError:  # pragma: no cover - path fallback for bare containers
    sys.path.insert(0, "/opt/trn_rl_repo")
    import concourse.bass as bass

import concourse.bacc as bacc
import concourse.tile as tile
from concourse import mybir
from concourse.bass_utils import run_bass_kernel_spmd

B, C, HH, WW = 4, 512, 32, 32
N = HH * WW          # 1024 spatial positions (= keys)
NQ = N // 2          # queries per core
NH, HD = 8, 64       # heads, head dim
P = 128
CT = C // P          # 4 channel partition-tiles
NKT = N // P         # 8 key tiles
NCORES = 8
BF16 = mybir.dt.bfloat16
FP32 = mybir.dt.float32
NP_BF16 = ml_dtypes.bfloat16
E = HD + 1           # head channels + ones column in vd tiles


# --------------------------------------------------------------------------
# host-side helpers
# --------------------------------------------------------------------------

def _gather_T(offsets_b, salf_b):
    """GT[k, n]: weight of source pixel k in grid-sampled output pixel n,
    with the per-source value modulation salf folded in. fp32 [N, N]."""
    ys = np.linspace(-1.0, 1.0, HH)
    xs = np.linspace(-1.0, 1.0, WW)
    gy, gx = np.meshgrid(ys, xs, indexing="ij")
    x = ((gx + offsets_b[0] / (WW / 2.0) + 1.0) * WW - 1.0) * 0.5
    y = ((gy + offsets_b[1] / (HH / 2.0) + 1.0) * HH - 1.0) * 0.5
    x = np.clip(x, 0.0, WW - 1.0)
    y = np.clip(y, 0.0, HH - 1.0)
    x0 = np.floor(x); y0 = np.floor(y)
    wx = x - x0; wy = y - y0
    x0i = x0.astype(np.int64); y0i = y0.astype(np.int64)
    x1i = np.minimum(x0i + 1, WW - 1); y1i = np.minimum(y0i + 1, HH - 1)
    GT = np.zeros((N, N), np.float32)
    n_idx = np.arange(N)
    for yi, xi, w in ((y0i, x0i, (1 - wx) * (1 - wy)),
                      (y0i, x1i, wx * (1 - wy)),
                      (y1i, x0i, (1 - wx) * wy),
                      (y1i, x1i, wx * wy)):
        np.add.at(GT, ((yi * WW + xi).reshape(-1), n_idx),
                  w.reshape(-1).astype(np.float32))
    GT *= salf_b[:, None]
    return GT


def _reference_numpy(q_feat, kv_feat, offsets, saliency_map, P_thermal,
                     Wq, bq, Wk, bk, Wv, bv, Wo, bo, lambda_p, gamma_val):
    """Plain numpy port of reference.py -- correctness fallback only."""
    qf = q_feat.reshape(B, C, N).transpose(0, 2, 1)
    kf = kv_feat.reshape(B, C, N).transpose(0, 2, 1)

    def heads(x, Wm, bm):
        return (x @ Wm.T + bm).reshape(B, N, NH, HD).transpose(0, 2, 1, 3)

    Q = heads(qf, Wq, bq)
    K = heads(kf, Wk, bk)
    V = heads(kf, Wv, bv)
    attn = np.einsum("bhqd,bhkd->bhqk", Q, K) * (HD ** -0.5)
    attn = attn + float(lambda_p) * P_thermal.reshape(B, 1, N, 1)
    attn = attn - attn.max(axis=-1, keepdims=True)
    w = np.exp(attn)
    w /= w.sum(axis=-1, keepdims=True)
    Vm = V * (1.0 + float(gamma_val) * saliency_map.reshape(B, 1, N, 1))
    Vsp = Vm.transpose(0, 2, 1, 3).reshape(B, N, C).transpose(0, 2, 1)
    Vd = np.empty_like(Vsp)
    for b in range(B):
        GT = _gather_T(offsets[b], np.ones(N, np.float32))
        Vd[b] = Vsp[b] @ GT
    Vdf = Vd.reshape(B, C, N).transpose(0, 2, 1).reshape(B, N, NH, HD).transpose(0, 2, 1, 3)
    out = np.einsum("bhqk,bhkd->bhqd", w, Vdf)
    out = out.transpose(0, 2, 1, 3).reshape(B, N, C)
    out = out @ Wo.T + bo
    return out.transpose(0, 2, 1).reshape(B, C, HH, WW).astype(np.float32)


# --------------------------------------------------------------------------
# device program
# --------------------------------------------------------------------------

def _build_program(chunks):
    """chunks: ordered list of (m, k) gather-tile pairs; same for all cores."""
    nch = len(chunks)
    chunks_for_m = {m: [] for m in range(NKT)}
    for idx, (m, k) in enumerate(chunks):
        chunks_for_m[m].append((idx, k))

    nc = bacc.Bacc(None, target_bir_lowering=False, debug=False)
    xq_d = nc.declare_dram_parameter("xq", [C, NQ], BF16, isOutput=False)
    xkv_d = nc.declare_dram_parameter("xkv", [C, N], BF16, isOutput=False)
    wq_d = nc.declare_dram_parameter("wqT", [C, C], BF16, isOutput=False)
    wk_d = nc.declare_dram_parameter("wkT", [C, C], BF16, isOutput=False)
    wv_d = nc.declare_dram_parameter("wvT", [C, C], BF16, isOutput=False)
    wo_d = nc.declare_dram_parameter("woT", [C, C], BF16, isOutput=False)
    gt_d = nc.declare_dram_parameter("gt", [nch, P, P], BF16, isOutput=False)
    out_d = nc.declare_dram_parameter("outT", [C, NQ], FP32, isOutput=True)

    with tile.TileContext(nc) as tc:
        with tc.tile_pool(name="const", bufs=1) as const, \
             tc.tile_pool(name="work", bufs=1) as work, \
             tc.tile_pool(name="pu_pool", bufs=1) as pu_pool, \
             tc.tile_pool(name="sm", bufs=4) as sm, \
             tc.tile_pool(name="psp", bufs=2, space="PSUM") as psp:

            # ---- fat input DMAs: one transfer per tensor, spread over the
            # four engine queues in consumption order.
            def fat_load(engine, dram, nm, width, row_lo=0, rows=C):
                """Load dram[row_lo:row_lo+rows, :width] as [128, rows//128 * width]."""
                kt_ = rows // P
                tl = const.tile([P, kt_ * width], BF16, name=nm, tag=nm)
                engine.dma_start(
                    out=tl[:].rearrange("p (k n) -> p k n", n=width),
                    in_=dram[row_lo:row_lo + rows, :].rearrange(
                        "(k p) n -> p k n", p=P))
                return tl

            # Only sync (SP), scalar (ACT) and gpsimd queues can issue DMAs.
            # criticality order per queue (~95GB/s each):
            #   gpsimd: xkv[0:128], xkv[128:256], wk
            #   sync:   wq, xkv[384:512], wv, wo
            #   scalar: xq, xkv[256:384], gt
            wk_w = fat_load(nc.gpsimd, wk_d, "wkw", C)
            wq_w = fat_load(nc.sync, wq_d, "wqw", C)
            xq_w = fat_load(nc.scalar, xq_d, "xqw", NQ)
            xkv_t = [None] * CT
            xkv_t[0] = fat_load(nc.gpsimd, xkv_d, "xkv0", N, row_lo=0, rows=P)
            xkv_t[3] = fat_load(nc.sync, xkv_d, "xkv3", N, row_lo=3 * P, rows=P)
            xkv_t[2] = fat_load(nc.scalar, xkv_d, "xkv2", N, row_lo=2 * P, rows=P)
            xkv_t[1] = fat_load(nc.gpsimd, xkv_d, "xkv1", N, row_lo=P, rows=P)
            wv_w = fat_load(nc.sync, wv_d, "wvw", C)
            gt_w = const.tile([P, nch * P], BF16, name="gtw", tag="gtw")
            nc.scalar.dma_start(
                out=gt_w[:].rearrange("p (c j) -> p c j", j=P),
                in_=gt_d[:].rearrange("c p j -> p c j"))
            wo_w = fat_load(nc.sync, wo_d, "wow", C)

            def wq_s(k, t):
                return wq_w[:, k * C + t * P: k * C + (t + 1) * P]

            def wk_s(k, t):
                return wk_w[:, k * C + t * P: k * C + (t + 1) * P]

            def xq_s(k):
                return xq_w[:, k * NQ:(k + 1) * NQ]

            def xkv_s(k, lo, width):
                return xkv_t[k][:, lo: lo + width]

            def wv_s(k):
                return wv_w[:, k * C:(k + 1) * C]

            # ---- SBUF result tiles ---------------------------------------
            # qt pairs: qt_sb[a][:, (t%2)*NQ:] = head-pair t   (t = 2a, 2a+1)
            qt_sb = [work.tile([P, 2 * NQ], BF16, name=f"qt{a}", tag=f"qt{a}")
                     for a in range(2)]
            kt_sb = [work.tile([P, N], BF16, name=f"kt{t}", tag=f"kt{t}")
                     for t in range(CT)]
            # v pairs: v_sb[j][:, (m%2)*C:] = keys tile m      (m = 2j, 2j+1)
            v_sb = [work.tile([P, 2 * C], BF16, name=f"v{j}", tag=f"v{j}")
                    for j in range(4)]
            # vd pairs: per m a [P, NH*E] block; cols h*E..h*E+63 head h,
            # col h*E+64 = 1.0 (softmax denominator trick)
            vd_sb = [work.tile([P, 2 * NH * E], BF16, name=f"vd{j}",
                               tag=f"vd{j}") for j in range(4)]
            o_pair = [work.tile([P, NQ], BF16, name=f"op{p}", tag=f"op{p}")
                      for p in range(CT)]
            pu_tiles = {}
            ps_o = [None] * CT

            def vd_l(m, h):
                j, r = m // 2, m % 2
                return vd_sb[j][:, r * NH * E + h * E: r * NH * E + (h + 1) * E]

            # ones columns of the vd tiles (written once, up front)
            for j in range(4):
                v3 = vd_sb[j][:].rearrange("p (rh e) -> p rh e", e=E)
                nc.vector.memset(v3[:, :, HD:E], 1.0)

            # ---- emission helpers ----------------------------------------
            def emit_qt_pair(a):
                ps = psp.tile([P, 2 * NQ], FP32, name=f"psq{a}", tag="acc",
                              bufs=2)
                for t in (2 * a, 2 * a + 1):
                    for k in range(CT):
                        nc.tensor.matmul(ps[:, (t % 2) * NQ:(t % 2 + 1) * NQ],
                                         lhsT=wq_s(k, t), rhs=xq_s(k),
                                         start=(k == 0), stop=(k == CT - 1))
                nc.vector.tensor_copy(qt_sb[a][:], ps[:])

            def emit_kt_half(t, half, ps):
                # k-order matches xkv tile DMA arrival
                for i, k in enumerate((2, 3, 0, 1)):
                    nc.tensor.matmul(ps[:, half * NQ:(half + 1) * NQ],
                                     lhsT=wk_s(k, t),
                                     rhs=xkv_s(k, half * NQ, NQ),
                                     start=(i == 0), stop=(i == CT - 1))

            def evac_kt_half(t, half, ps):
                nc.vector.tensor_copy(
                    kt_sb[t][:, half * NQ:(half + 1) * NQ],
                    ps[:, half * NQ:(half + 1) * NQ])

            def emit_s_chunk(t, m):
                ps_s = psp.tile([P, 2 * NQ], FP32, name=f"pss{t}_{m}",
                                tag="ps_s", bufs=2)
                kt, qt = kt_sb[t], qt_sb[t // 2]
                qo = (t % 2) * NQ
                nc.tensor.matmul(ps_s[:, 0:NQ],
                                 lhsT=kt[0:HD, m * P:(m + 1) * P],
                                 rhs=qt[0:HD, qo:qo + NQ],
                                 start=True, stop=True)
                nc.tensor.matmul(ps_s[:, NQ:2 * NQ],
                                 lhsT=kt[HD:P, m * P:(m + 1) * P],
                                 rhs=qt[HD:P, qo:qo + NQ],
                                 start=True, stop=True)
                pu = pu_pool.tile([P, 2 * NQ], BF16, name=f"pu{t}_{m}",
                                  tag=f"pu{t}_{m}")
                nc.scalar.activation(out=pu[:], in_=ps_s[:],
                                     func=mybir.ActivationFunctionType.Exp)
                pu_tiles[(t, m)] = pu

            def emit_v_pair(j, ps):
                for m in (2 * j, 2 * j + 1):
                    for k in range(CT):
                        nc.tensor.matmul(ps[:, (m % 2) * C:(m % 2 + 1) * C],
                                         lhsT=xkv_s(k, m * P, P),
                                         rhs=wv_s(k),
                                         start=(k == 0), stop=(k == CT - 1))
                nc.vector.tensor_copy(v_sb[j][:], ps[:])

            def emit_vd_pair(j, ps):
                for m in (2 * j, 2 * j + 1):
                    lst = chunks_for_m[m]
                    for i, (idx, k) in enumerate(lst):
                        nc.tensor.matmul(
                            ps[:, (m % 2) * C:(m % 2 + 1) * C],
                            lhsT=gt_w[:, idx * P:(idx + 1) * P],
                            rhs=v_sb[k // 2][:, (k % 2) * C:(k % 2 + 1) * C],
                            start=(i == 0), stop=(i == len(lst) - 1))
                # strided evacuation into the [.., h, 0:64] sub-columns
                dst = vd_sb[j][:].rearrange("p (r h e) -> p r h e", e=E, h=NH)
                src = ps[:].rearrange("p (r h e) -> p r h e", e=HD, h=NH)
                nc.vector.tensor_copy(dst[:, :, :, 0:HD], src[:])

            def emit_o_chunk(hp, m, tag="acc"):
                # head pairs 2,3 accumulate in the ps_s slots (free once the
                # exp stream drains) so they never wait on pair 0/1 norms
                if ps_o[hp] is None:
                    ps_o[hp] = psp.tile([P, 2 * NQ], FP32, name=f"pso{hp}",
                                        tag=tag, bufs=2)
                ps = ps_o[hp]
                pu = pu_tiles[(hp, m)]
                nc.tensor.matmul(ps[0:E, 0:NQ],
                                 lhsT=vd_l(m, 2 * hp), rhs=pu[:, 0:NQ],
                                 start=(m == 0), stop=(m == NKT - 1))
                nc.tensor.matmul(ps[0:E, NQ:2 * NQ],
                                 lhsT=vd_l(m, 2 * hp + 1), rhs=pu[:, NQ:2 * NQ],
                                 start=(m == 0), stop=(m == NKT - 1))

            def emit_norm(hp):
                # heads 2hp (cols 0:NQ) and 2hp+1 (cols NQ:2NQ) of ps_o[hp]
                ps = ps_o[hp]
                for hh in range(2):
                    lo = hh * NQ
                    rec = sm.tile([1, NQ], FP32, name=f"rec{hp}_{hh}",
                                  tag=f"rec{hp}_{hh}", bufs=1)
                    nc.vector.reciprocal(rec[:], ps[HD:E, lo:lo + NQ])
                    bc = sm.tile([HD, NQ], FP32, name=f"bc{hp}_{hh}",
                                 tag="bc", bufs=4)
                    nc.gpsimd.partition_broadcast(bc[:], rec[:])
                    nc.vector.tensor_mul(
                        o_pair[hp][hh * HD:(hh + 1) * HD, :],
                        ps[0:HD, lo:lo + NQ], bc[:])

            # ---- emission schedule ---------------------------------------
            # warmup: KT(0) first (longest DMA dependency chain), QT pairs
            # as filler, then the exp-paced S stream with V/Vd/O interleave.
            # PSUM tiles are allocated strictly in use order so the acc-tag
            # round-robin (bufs=2) matches the schedule.
            emit_qt_pair(0)
            psk0 = psp.tile([P, 2 * NQ], FP32, name="psk0", tag="acc", bufs=2)
            emit_kt_half(0, 0, psk0)
            evac_kt_half(0, 0, psk0)
            emit_kt_half(0, 1, psk0)
            evac_kt_half(0, 1, psk0)
            emit_s_chunk(0, 0)
            emit_s_chunk(0, 1)
            # KT(1..3) + QT pair 1 interleaved into the early S stream
            for t in (1, 2, 3):
                pskt = psp.tile([P, 2 * NQ], FP32, name=f"psk{t}", tag="acc",
                                bufs=2)
                for half in range(2):
                    emit_kt_half(t, half, pskt)
                    evac_kt_half(t, half, pskt)
                    emit_s_chunk(0, 2 * t + half)
                if t == 1:
                    emit_qt_pair(1)
            # t=1 stream: one V pair per two exp slots
            for j in range(4):
                ps = psp.tile([P, 2 * C], FP32, name=f"psv{j}", tag="acc",
                              bufs=2)
                emit_s_chunk(1, 2 * j)
                emit_v_pair(j, ps)
                emit_s_chunk(1, 2 * j + 1)
            # t=2 stream: Vd pairs
            for j in range(4):
                ps = psp.tile([P, 2 * C], FP32, name=f"psvd{j}", tag="acc",
                              bufs=2)
                emit_s_chunk(2, 2 * j)
                emit_vd_pair(j, ps)
                emit_s_chunk(2, 2 * j + 1)
            # t=3 stream: O chunks for head pairs 0 and 1
            for m in range(NKT):
                emit_s_chunk(3, m)
                emit_o_chunk(0, m)
                emit_o_chunk(1, m)
            emit_norm(0)
            emit_norm(1)
            for m in range(NKT):
                emit_o_chunk(2, m, tag="ps_s")
            emit_norm(2)
            for m in range(NKT):
                emit_o_chunk(3, m, tag="ps_s")
            emit_norm(3)

            # ---- out^T = Wo @ O : [C, NQ] fp32 ---------------------------
            # K=128 GEMM over the four normalized pair tiles; per 128-row
            # output tile: accumulate, evacuate, DMA in two row-halves
            # spread over the three DMA-capable queues.
            out_q = [(nc.sync, nc.scalar), (nc.gpsimd, nc.sync),
                     (nc.scalar, nc.gpsimd), (nc.sync, nc.scalar)]
            for th in range(2):
                ps = psp.tile([P, 2 * NQ], FP32, name=f"psw{th}", tag="acc",
                              bufs=2)
                for t in (2 * th, 2 * th + 1):
                    for p in range(CT):
                        nc.tensor.matmul(
                            ps[:, (t % 2) * NQ:(t % 2 + 1) * NQ],
                            lhsT=wo_w[:, p * C + t * P: p * C + (t + 1) * P],
                            rhs=o_pair[p][:],
                            start=(p == 0), stop=(p == CT - 1))
                    obt = sm.tile([P, NQ], FP32, name=f"ob{t}", tag="ob",
                                  bufs=2)
                    nc.vector.tensor_copy(
                        obt[:], ps[:, (t % 2) * NQ:(t % 2 + 1) * NQ])
                    qa, qb = out_q[t]
                    qa.dma_start(out=out_d[t * P:t * P + 64, :],
                                 in_=obt[0:64, :])
                    qb.dma_start(out=out_d[t * P + 64:(t + 1) * P, :],
                                 in_=obt[64:P, :])

    nc.compile()
    return nc


# --------------------------------------------------------------------------
# public entry points
# --------------------------------------------------------------------------

def _prepare(inputs):
    q = np.ascontiguousarray(inputs["q_feat"], np.float32).reshape(B, C, N)
    kv = np.ascontiguousarray(inputs["kv_feat"], np.float32).reshape(B, C, N)
    offsets = np.asarray(inputs["offsets"], np.float32)
    sal = np.asarray(inputs["saliency_map"], np.float32).reshape(B, N)
    gamma = float(np.asarray(inputs["gamma_val"]))

    GTs = [_gather_T(offsets[b], 1.0 + gamma * sal[b]) for b in range(B)]

    # union band-sparsity pattern of the gather matmul across batches, so the
    # SPMD program is identical on every core
    chunks = []
    for m in range(NKT):
        for k in range(NKT):
            if any(GTs[b][k * P:(k + 1) * P, m * P:(m + 1) * P].any()
                   for b in range(B)):
                chunks.append((m, k))

    wqT = np.ascontiguousarray((np.asarray(inputs["Wq"], np.float32).T
                                * (HD ** -0.5)).astype(NP_BF16))
    wkT = np.ascontiguousarray(np.asarray(inputs["Wk"], np.float32).T.astype(NP_BF16))
    wvT = np.ascontiguousarray(np.asarray(inputs["Wv"], np.float32).T.astype(NP_BF16))
    woT = np.ascontiguousarray(np.asarray(inputs["Wo"], np.float32).T.astype(NP_BF16))

    in_maps = []
    for core in range(NCORES):
        b, qh = core // 2, core % 2
        gt_stack = np.stack([
            np.ascontiguousarray(
                GTs[b][k * P:(k + 1) * P, m * P:(m + 1) * P]).astype(NP_BF16)
            for (m, k) in chunks])
        in_maps.append({
            "xq": np.ascontiguousarray(
                q[b][:, qh * NQ:(qh + 1) * NQ]).astype(NP_BF16),
            "xkv": np.ascontiguousarray(kv[b]).astype(NP_BF16),
            "wqT": wqT, "wkT": wkT, "wvT": wvT, "woT": woT,
            "gt": gt_stack,
        })

    def assemble(results):
        out = np.empty((B, C, N), np.float32)
        for core in range(NCORES):
            b, qh = core // 2, core % 2
            out[b][:, qh * NQ:(qh + 1) * NQ] = results[core]["outT"]
        return out.reshape(B, C, HH, WW)

    nc = _build_program(chunks)
    return nc, in_maps, assemble


def _needs_fallback(inputs):
    try:
        if tuple(np.shape(inputs["q_feat"])) != (B, C, HH, WW):
            return True
        for bias in ("bq", "bk", "bv", "bo"):
            if np.any(np.asarray(inputs[bias], np.float32) != 0.0):
                return True
    except Exception:
        return True
    return False


def kernel(**inputs) -> np.ndarray:
    if _needs_fallback(inputs):
        return _reference_numpy(**{k: np.asarray(v, np.float32)
                                   for k, v in inputs.items()})
    nc, in_maps, assemble = _prepare(inputs)
    res = run_bass_kernel_spmd(nc, in_maps, core_ids=list(range(NCORES)))
    return assemble(res.results)


def kernel_traced(trace_cores=(0,), **inputs):
    """Like kernel() but returns (output, exec_time_ns, trace_path)."""
    nc, in_maps, assemble = _prepare(inputs)
    res = run_bass_kernel_spmd(nc, in_maps, core_ids=list(range(NCORES)),
                               trace=True, trace_cores=list(trace_cores))
    trace_path = None
    if res.instructions_and_trace is not None:
        trace_path = res.instructions_and_trace[1]
    return assemble(res.results), res.exec_time_ns, trace_path


# revision 15
# speedup vs baseline: 1.1526x; 1.1526x over previous
"""Trainium2 Bass kernel: DeformableValueAttention.

Full-input contract: kernel(**inputs) takes the unsharded inputs of
reference.setup_inputs() and returns the full [B, C, H, W] output.

Sharding: 8 cores = (batch b, query-half qh). Each core computes attention
for 512 queries x all 1024 keys of one batch and produces a disjoint
[C, 512] column-slice of the output -- no cross-core reduction.

Per-core algorithm (channels-on-partitions layouts):
  QT = (Wq/8) @ xq          [C, Nq]
  KT = Wk @ xkv             [C, N]
  V  = xkv^T @ Wv^T         [N, C]    (keys on partitions)
  Vd = G^T.T @ V            [N, C]    grid_sample as banded sparse matmul,
                                      (1+gamma*sal) folded into G
  per head pair t (2-head PE row-group packing, K=64):
    S^T = KT_t^T @ QT_t     [N, 2*Nq] scores, both heads side by side
    Pu  = exp(S^T)          bf16      (no max-subtraction: |scores| < ~6)
    O   = [Vd_h | 1]^T @ Pu [65, Nq]  per head; ones-column gives the
                                      softmax denominator in row 64
  normalize: DVE reciprocal of denom row + GPSIMD partition_broadcast +
             DVE multiply (no ACT -> no activation-table ping-pong);
             normalized heads land pairwise in [128, Nq] channel tiles
  out^T = Wo @ O            [C, Nq]   K=128 GEMM over the 4 pair tiles

Schedule notes (static in-order engine queues -> emission order is the
schedule):
  - Inputs arrive via one fat DMA per tensor, spread over the 4 engine
    queues in criticality order so the first exp fires ~6us earlier.
  - The 32 exp activations are the ACT pacer (~1.1us each); V, Vd and the
    first half of the O matmuls are interleaved into the S stream so the
    PE stays busy under the exp stream.
  - PSUM: two [128,1024] tags x 2 bufs = all 8 banks. O accumulates head A
    in bank cols 0:512 and head B in 512:1024 of one tile.
  - Output DMA is split per 128-row tile across all 4 queues.

Fidelity notes:
  - P_thermal adds a per-query constant pre-softmax; softmax is invariant.
  - Zero biases assumed (asserted); numpy fallback otherwise.
  - bf16 matmuls, fp32 PSUM; measured rel err ~5e-3 vs fp32 reference.
"""

import os
import sys

import numpy as np
import ml_dtypes

try:
    import concourse.bass as bass
except ImportError:  # pragma: no cover - path fallback for bare containers
    sys.path.insert(0, "/opt/trn_rl_repo")
    import concourse.bass as bass

import concourse.bacc as bacc
import concourse.tile as tile
from concourse import mybir
from concourse.bass_utils import run_bass_kernel_spmd

B, C, HH, WW = 4, 512, 32, 32
N = HH * WW          # 1024 spatial positions (= keys)
NQ = N // 2          # queries per core
NH, HD = 8, 64       # heads, head dim
P = 128
CT = C // P          # 4 channel partition-tiles
NKT = N // P         # 8 key tiles
NCORES = 8
BF16 = mybir.dt.bfloat16
FP32 = mybir.dt.float32
NP_BF16 = ml_dtypes.bfloat16
E = HD + 1           # head channels + ones column in vd tiles


# --------------------------------------------------------------------------
# host-side helpers
# --------------------------------------------------------------------------

def _gather_T(offsets_b, salf_b):
    """GT[k, n]: weight of source pixel k in grid-sampled output pixel n,
    with the per-source value modulation salf folded in. fp32 [N, N]."""
    ys = np.linspace(-1.0, 1.0, HH)
    xs = np.linspace(-1.0, 1.0, WW)
    gy, gx = np.meshgrid(ys, xs, indexing="ij")
    x = ((gx + offsets_b[0] / (WW / 2.0) + 1.0) * WW - 1.0) * 0.5
    y = ((gy + offsets_b[1] / (HH / 2.0) + 1.0) * HH - 1.0) * 0.5
    x = np.clip(x, 0.0, WW - 1.0)
    y = np.clip(y, 0.0, HH - 1.0)
    x0 = np.floor(x); y0 = np.floor(y)
    wx = x - x0; wy = y - y0
    x0i = x0.astype(np.int64); y0i = y0.astype(np.int64)
    x1i = np.minimum(x0i + 1, WW - 1); y1i = np.minimum(y0i + 1, HH - 1)
    GT = np.zeros((N, N), np.float32)
    n_idx = np.arange(N)
    for yi, xi, w in ((y0i, x0i, (1 - wx) * (1 - wy)),
                      (y0i, x1i, wx * (1 - wy)),
                      (y1i, x0i, (1 - wx) * wy),
                      (y1i, x1i, wx * wy)):
        np.add.at(GT, ((yi * WW + xi).reshape(-1), n_idx),
                  w.reshape(-1).astype(np.float32))
    GT *= salf_b[:, None]
    return GT


def _reference_numpy(q_feat, kv_feat, offsets, saliency_map, P_thermal,
                     Wq, bq, Wk, bk, Wv, bv, Wo, bo, lambda_p, gamma_val):
    """Plain numpy port of reference.py -- correctness fallback only."""
    qf = q_feat.reshape(B, C, N).transpose(0, 2, 1)
    kf = kv_feat.reshape(B, C, N).transpose(0, 2, 1)

    def heads(x, Wm, bm):
        return (x @ Wm.T + bm).reshape(B, N, NH, HD).transpose(0, 2, 1, 3)

    Q = heads(qf, Wq, bq)
    K = heads(kf, Wk, bk)
    V = heads(kf, Wv, bv)
    attn = np.einsum("bhqd,bhkd->bhqk", Q, K) * (HD ** -0.5)
    attn = attn + float(lambda_p) * P_thermal.reshape(B, 1, N, 1)
    attn = attn - attn.max(axis=-1, keepdims=True)
    w = np.exp(attn)
    w /= w.sum(axis=-1, keepdims=True)
    Vm = V * (1.0 + float(gamma_val) * saliency_map.reshape(B, 1, N, 1))
    Vsp = Vm.transpose(0, 2, 1, 3).reshape(B, N, C).transpose(0, 2, 1)
    Vd = np.empty_like(Vsp)
    for b in range(B):
        GT = _gather_T(offsets[b], np.ones(N, np.float32))
        Vd[b] = Vsp[b] @ GT
    Vdf = Vd.reshape(B, C, N).transpose(0, 2, 1).reshape(B, N, NH, HD).transpose(0, 2, 1, 3)
    out = np.einsum("bhqk,bhkd->bhqd", w, Vdf)
    out = out.transpose(0, 2, 1, 3).reshape(B, N, C)
    out = out @ Wo.T + bo
    return out.transpose(0, 2, 1).reshape(B, C, HH, WW).astype(np.float32)


# --------------------------------------------------------------------------
# device program
# --------------------------------------------------------------------------

def _build_program(chunks):
    """chunks: ordered list of (m, k) gather-tile pairs; same for all cores."""
    nch = len(chunks)
    chunks_for_m = {m: [] for m in range(NKT)}
    for idx, (m, k) in enumerate(chunks):
        chunks_for_m[m].append((idx, k))

    nc = bacc.Bacc(None, target_bir_lowering=False, debug=False)
    xq_d = nc.declare_dram_parameter("xq", [C, NQ], BF16, isOutput=False)
    xkv_d = nc.declare_dram_parameter("xkv", [C, N], BF16, isOutput=False)
    wq_d = nc.declare_dram_parameter("wqT", [C, C], BF16, isOutput=False)
    wk_d = nc.declare_dram_parameter("wkT", [C, C], BF16, isOutput=False)
    wv_d = nc.declare_dram_parameter("wvT", [C, C], BF16, isOutput=False)
    wo_d = nc.declare_dram_parameter("woT", [C, C], BF16, isOutput=False)
    gt_d = nc.declare_dram_parameter("gt", [nch, P, P], BF16, isOutput=False)
    out_d = nc.declare_dram_parameter("outT", [C, NQ], FP32, isOutput=True)

    with tile.TileContext(nc) as tc:
        with tc.tile_pool(name="const", bufs=1) as const, \
             tc.tile_pool(name="work", bufs=1) as work, \
             tc.tile_pool(name="pu_pool", bufs=1) as pu_pool, \
             tc.tile_pool(name="sm", bufs=4) as sm, \
             tc.tile_pool(name="psp", bufs=2, space="PSUM") as psp:

            # ---- fat input DMAs: one transfer per tensor, spread over the
            # four engine queues in consumption order.
            def fat_load(engine, dram, nm, width, row_lo=0, rows=C):
                """Load dram[row_lo:row_lo+rows, :width] as [128, rows//128 * width]."""
                kt_ = rows // P
                tl = const.tile([P, kt_ * width], BF16, name=nm, tag=nm)
                engine.dma_start(
                    out=tl[:].rearrange("p (k n) -> p k n", n=width),
                    in_=dram[row_lo:row_lo + rows, :].rearrange(
                        "(k p) n -> p k n", p=P))
                return tl

            # Only sync (SP), scalar (ACT) and gpsimd queues can issue DMAs.
            # criticality order per queue (~95GB/s each):
            #   gpsimd: xkv[0:128], xkv[128:256], wk
            #   sync:   wq, xkv[384:512], wv, wo
            #   scalar: xq, xkv[256:384], gt
            wk_w = fat_load(nc.gpsimd, wk_d, "wkw", C)
            wq_w = fat_load(nc.sync, wq_d, "wqw", C)
            xq_w = fat_load(nc.scalar, xq_d, "xqw", NQ)
            xkv_t = [None] * CT
            xkv_t[0] = fat_load(nc.gpsimd, xkv_d, "xkv0", N, row_lo=0, rows=P)
            xkv_t[3] = fat_load(nc.sync, xkv_d, "xkv3", N, row_lo=3 * P, rows=P)
            xkv_t[2] = fat_load(nc.scalar, xkv_d, "xkv2", N, row_lo=2 * P, rows=P)
            xkv_t[1] = fat_load(nc.gpsimd, xkv_d, "xkv1", N, row_lo=P, rows=P)
            wv_w = fat_load(nc.sync, wv_d, "wvw", C)
            gt_w = const.tile([P, nch * P], BF16, name="gtw", tag="gtw")
            nc.scalar.dma_start(
                out=gt_w[:].rearrange("p (c j) -> p c j", j=P),
                in_=gt_d[:].rearrange("c p j -> p c j"))
            wo_w = fat_load(nc.sync, wo_d, "wow", C)

            def wq_s(k, t):
                return wq_w[:, k * C + t * P: k * C + (t + 1) * P]

            def wk_s(k, t):
                return wk_w[:, k * C + t * P: k * C + (t + 1) * P]

            def xq_s(k):
                return xq_w[:, k * NQ:(k + 1) * NQ]

            def xkv_s(k, lo, width):
                return xkv_t[k][:, lo: lo + width]

            def wv_s(k):
                return wv_w[:, k * C:(k + 1) * C]

            # ---- SBUF result tiles ---------------------------------------
            # qt pairs: qt_sb[a][:, (t%2)*NQ:] = head-pair t   (t = 2a, 2a+1)
            qt_sb = [work.tile([P, 2 * NQ], BF16, name=f"qt{a}", tag=f"qt{a}")
                     for a in range(2)]
            kt_sb = [work.tile([P, N], BF16, name=f"kt{t}", tag=f"kt{t}")
                     for t in range(CT)]
            # v pairs: v_sb[j][:, (m%2)*C:] = keys tile m      (m = 2j, 2j+1)
            v_sb = [work.tile([P, 2 * C], BF16, name=f"v{j}", tag=f"v{j}")
                    for j in range(4)]
            # vd pairs: per m a [P, NH*E] block; cols h*E..h*E+63 head h,
            # col h*E+64 = 1.0 (softmax denominator trick)
            vd_sb = [work.tile([P, 2 * NH * E], BF16, name=f"vd{j}",
                               tag=f"vd{j}") for j in range(4)]
            o_pair = [work.tile([P, NQ], BF16, name=f"op{p}", tag=f"op{p}")
                      for p in range(CT)]
            pu_tiles = {}
            ps_o = [None] * CT

            def vd_l(m, h):
                j, r = m // 2, m % 2
                return vd_sb[j][:, r * NH * E + h * E: r * NH * E + (h + 1) * E]

            # ones columns of the vd tiles (written once, up front)
            for j in range(4):
                v3 = vd_sb[j][:].rearrange("p (rh e) -> p rh e", e=E)
                nc.vector.memset(v3[:, :, HD:E], 1.0)

            # ---- emission helpers ----------------------------------------
            def emit_qt_pair(a):
                ps = psp.tile([P, 2 * NQ], FP32, name=f"psq{a}", tag="acc",
                              bufs=2)
                for t in (2 * a, 2 * a + 1):
                    for k in range(CT):
                        nc.tensor.matmul(ps[:, (t % 2) * NQ:(t % 2 + 1) * NQ],
                                         lhsT=wq_s(k, t), rhs=xq_s(k),
                                         start=(k == 0), stop=(k == CT - 1))
                nc.vector.tensor_copy(qt_sb[a][:], ps[:])

            def emit_kt_half(t, half, ps):
                # k-order matches xkv tile DMA arrival
                for i, k in enumerate((2, 3, 0, 1)):
                    nc.tensor.matmul(ps[:, half * NQ:(half + 1) * NQ],
                                     lhsT=wk_s(k, t),
                                     rhs=xkv_s(k, half * NQ, NQ),
                                     start=(i == 0), stop=(i == CT - 1))

            def evac_kt_half(t, half, ps):
                nc.vector.tensor_copy(
                    kt_sb[t][:, half * NQ:(half + 1) * NQ],
                    ps[:, half * NQ:(half + 1) * NQ])

            def emit_s_chunk(t, m):
                ps_s = psp.tile([P, 2 * NQ], FP32, name=f"pss{t}_{m}",
                                tag="ps_s", bufs=2)
                kt, qt = kt_sb[t], qt_sb[t // 2]
                qo = (t % 2) * NQ
                nc.tensor.matmul(ps_s[:, 0:NQ],
                                 lhsT=kt[0:HD, m * P:(m + 1) * P],
                                 rhs=qt[0:HD, qo:qo + NQ],
                                 start=True, stop=True)
                nc.tensor.matmul(ps_s[:, NQ:2 * NQ],
                                 lhsT=kt[HD:P, m * P:(m + 1) * P],
                                 rhs=qt[HD:P, qo:qo + NQ],
                                 start=True, stop=True)
                pu = pu_pool.tile([P, 2 * NQ], BF16, name=f"pu{t}_{m}",
                                  tag=f"pu{t}_{m}")
                nc.scalar.activation(out=pu[:], in_=ps_s[:],
                                     func=mybir.ActivationFunctionType.Exp)
                pu_tiles[(t, m)] = pu

            def emit_v_pair(j, ps):
                for m in (2 * j, 2 * j + 1):
                    for k in range(CT):
                        nc.tensor.matmul(ps[:, (m % 2) * C:(m % 2 + 1) * C],
                                         lhsT=xkv_s(k, m * P, P),
                                         rhs=wv_s(k),
                                         start=(k == 0), stop=(k == CT - 1))
                nc.vector.tensor_copy(v_sb[j][:], ps[:])

            def emit_vd_pair(j, ps):
                for m in (2 * j, 2 * j + 1):
                    lst = chunks_for_m[m]
                    for i, (idx, k) in enumerate(lst):
                        nc.tensor.matmul(
                            ps[:, (m % 2) * C:(m % 2 + 1) * C],
                            lhsT=gt_w[:, idx * P:(idx + 1) * P],
                            rhs=v_sb[k // 2][:, (k % 2) * C:(k % 2 + 1) * C],
                            start=(i == 0), stop=(i == len(lst) - 1))
                # strided evacuation into the [.., h, 0:64] sub-columns
                dst = vd_sb[j][:].rearrange("p (r h e) -> p r h e", e=E, h=NH)
                src = ps[:].rearrange("p (r h e) -> p r h e", e=HD, h=NH)
                nc.vector.tensor_copy(dst[:, :, :, 0:HD], src[:])

            def emit_o_chunk(hp, m, tag="acc"):
                # head pairs 2,3 accumulate in the ps_s slots (free once the
                # exp stream drains) so they never wait on pair 0/1 norms
                if ps_o[hp] is None:
                    ps_o[hp] = psp.tile([P, 2 * NQ], FP32, name=f"pso{hp}",
                                        tag=tag, bufs=2)
                ps = ps_o[hp]
                pu = pu_tiles[(hp, m)]
                nc.tensor.matmul(ps[0:E, 0:NQ],
                                 lhsT=vd_l(m, 2 * hp), rhs=pu[:, 0:NQ],
                                 start=(m == 0), stop=(m == NKT - 1))
                nc.tensor.matmul(ps[0:E, NQ:2 * NQ],
                                 lhsT=vd_l(m, 2 * hp + 1), rhs=pu[:, NQ:2 * NQ],
                                 start=(m == 0), stop=(m == NKT - 1))

            def emit_norm(hp):
                # heads 2hp (cols 0:NQ) and 2hp+1 (cols NQ:2NQ) of ps_o[hp]
                ps = ps_o[hp]
                for hh in range(2):
                    lo = hh * NQ
                    den = sm.tile([1, NQ], FP32, name=f"den{hp}_{hh}",
                                  tag=f"den{hp}_{hh}", bufs=1)
                    nc.vector.tensor_copy(den[:], ps[HD:E, lo:lo + NQ])
                    bc = sm.tile([HD, NQ], FP32, name=f"bc{hp}_{hh}",
                                 tag="bc", bufs=4)
                    nc.gpsimd.partition_broadcast(bc[:], den[:])
                    rec = sm.tile([HD, NQ], FP32, name=f"rec{hp}_{hh}",
                                  tag="rec", bufs=4)
                    # ~51 ULP approx; denominators are >= ~1 so the undefined
                    # edge cases (0/denorm/inf) cannot occur. The exact
                    # InstReciprocal measured 3.3us per call and serialized
                    # the whole output phase.
                    nc.vector.reciprocal_approx_fast(rec[:], bc[:])
                    nc.vector.tensor_mul(
                        o_pair[hp][hh * HD:(hh + 1) * HD, :],
                        ps[0:HD, lo:lo + NQ], rec[:])

            # ---- emission schedule ---------------------------------------
            # warmup: KT(0) first (longest DMA dependency chain), QT pairs
            # as filler, then the exp-paced S stream with V/Vd/O interleave.
            # PSUM tiles are allocated strictly in use order so the acc-tag
            # round-robin (bufs=2) matches the schedule.
            emit_qt_pair(0)
            psk0 = psp.tile([P, 2 * NQ], FP32, name="psk0", tag="acc", bufs=2)
            emit_kt_half(0, 0, psk0)
            evac_kt_half(0, 0, psk0)
            emit_kt_half(0, 1, psk0)
            evac_kt_half(0, 1, psk0)
            emit_s_chunk(0, 0)
            emit_s_chunk(0, 1)
            # KT(1..3) + QT pair 1 interleaved into the early S stream
            for t in (1, 2, 3):
                pskt = psp.tile([P, 2 * NQ], FP32, name=f"psk{t}", tag="acc",
                                bufs=2)
                for half in range(2):
                    emit_kt_half(t, half, pskt)
                    evac_kt_half(t, half, pskt)
                    emit_s_chunk(0, 2 * t + half)
                if t == 1:
                    emit_qt_pair(1)
            # t=1 stream: one V pair per two exp slots
            for j in range(4):
                ps = psp.tile([P, 2 * C], FP32, name=f"psv{j}", tag="acc",
                              bufs=2)
                emit_s_chunk(1, 2 * j)
                emit_v_pair(j, ps)
                emit_s_chunk(1, 2 * j + 1)
            # t=2 stream: Vd pairs
            for j in range(4):
                ps = psp.tile([P, 2 * C], FP32, name=f"psvd{j}", tag="acc",
                              bufs=2)
                emit_s_chunk(2, 2 * j)
                emit_vd_pair(j, ps)
                emit_s_chunk(2, 2 * j + 1)
            # t=3 stream: O chunks for head pairs 0 and 1
            for m in range(NKT):
                emit_s_chunk(3, m)
                emit_o_chunk(0, m)
                emit_o_chunk(1, m)
            emit_norm(0)
            emit_norm(1)
            for m in range(NKT):
                emit_o_chunk(2, m, tag="ps_s")
            emit_norm(2)
            for m in range(NKT):
                emit_o_chunk(3, m, tag="ps_s")
            emit_norm(3)

            # ---- out^T = Wo @ O : [C, NQ] fp32 ---------------------------
            # K=128 GEMM over the four normalized pair tiles; per 128-row
            # output tile: accumulate, evacuate, DMA in two row-halves
            # spread over the three DMA-capable queues.
            # p-outer emission: the 12 matmuls over pairs 0-2 run while the
            # last pair is still normalizing; only the four p=3 matmuls sit
            # behind the final norm.
            out_q = [(nc.sync, nc.scalar), (nc.gpsimd, nc.sync),
                     (nc.scalar, nc.gpsimd), (nc.sync, nc.scalar)]
            ps_w = [psp.tile([P, 2 * NQ], FP32, name=f"psw{th}", tag="acc",
                             bufs=2) for th in range(2)]
            for p in range(CT):
                for t in range(CT):
                    ps = ps_w[t // 2]
                    nc.tensor.matmul(
                        ps[:, (t % 2) * NQ:(t % 2 + 1) * NQ],
                        lhsT=wo_w[:, p * C + t * P: p * C + (t + 1) * P],
                        rhs=o_pair[p][:],
                        start=(p == 0), stop=(p == CT - 1))
                    if p == CT - 1:
                        obt = sm.tile([P, NQ], FP32, name=f"ob{t}", tag="ob",
                                      bufs=2)
                        nc.vector.tensor_copy(
                            obt[:], ps[:, (t % 2) * NQ:(t % 2 + 1) * NQ])
                        qa, qb = out_q[t]
                        qa.dma_start(out=out_d[t * P:t * P + 64, :],
                                     in_=obt[0:64, :])
                        qb.dma_start(out=out_d[t * P + 64:(t + 1) * P, :],
                                     in_=obt[64:P, :])

    nc.compile()
    return nc


# --------------------------------------------------------------------------
# public entry points
# --------------------------------------------------------------------------

def _prepare(inputs):
    q = np.ascontiguousarray(inputs["q_feat"], np.float32).reshape(B, C, N)
    kv = np.ascontiguousarray(inputs["kv_feat"], np.float32).reshape(B, C, N)
    offsets = np.asarray(inputs["offsets"], np.float32)
    sal = np.asarray(inputs["saliency_map"], np.float32).reshape(B, N)
    gamma = float(np.asarray(inputs["gamma_val"]))

    GTs = [_gather_T(offsets[b], 1.0 + gamma * sal[b]) for b in range(B)]

    # union band-sparsity pattern of the gather matmul across batches, so the
    # SPMD program is identical on every core
    chunks = []
    for m in range(NKT):
        for k in range(NKT):
            if any(GTs[b][k * P:(k + 1) * P, m * P:(m + 1) * P].any()
                   for b in range(B)):
                chunks.append((m, k))

    wqT = np.ascontiguousarray((np.asarray(inputs["Wq"], np.float32).T
                                * (HD ** -0.5)).astype(NP_BF16))
    wkT = np.ascontiguousarray(np.asarray(inputs["Wk"], np.float32).T.astype(NP_BF16))
    wvT = np.ascontiguousarray(np.asarray(inputs["Wv"], np.float32).T.astype(NP_BF16))
    woT = np.ascontiguousarray(np.asarray(inputs["Wo"], np.float32).T.astype(NP_BF16))

    in_maps = []
    for core in range(NCORES):
        b, qh = core // 2, core % 2
        gt_stack = np.stack([
            np.ascontiguousarray(
                GTs[b][k * P:(k + 1) * P, m * P:(m + 1) * P]).astype(NP_BF16)
            for (m, k) in chunks])
        in_maps.append({
            "xq": np.ascontiguousarray(
                q[b][:, qh * NQ:(qh + 1) * NQ]).astype(NP_BF16),
            "xkv": np.ascontiguousarray(kv[b]).astype(NP_BF16),
            "wqT": wqT, "wkT": wkT, "wvT": wvT, "woT": woT,
            "gt": gt_stack,
        })

    def assemble(results):
        out = np.empty((B, C, N), np.float32)
        for core in range(NCORES):
            b, qh = core // 2, core % 2
            out[b][:, qh * NQ:(qh + 1) * NQ] = results[core]["outT"]
        return out.reshape(B, C, HH, WW)

    nc = _build_program(chunks)
    return nc, in_maps, assemble


def _needs_fallback(inputs):
    try:
        if tuple(np.shape(inputs["q_feat"])) != (B, C, HH, WW):
            return True
        for bias in ("bq", "bk", "bv", "bo"):
            if np.any(np.asarray(inputs[bias], np.float32) != 0.0):
                return True
    except Exception:
        return True
    return False


def kernel(**inputs) -> np.ndarray:
    if _needs_fallback(inputs):
        return _reference_numpy(**{k: np.asarray(v, np.float32)
                                   for k, v in inputs.items()})
    nc, in_maps, assemble = _prepare(inputs)
    res = run_bass_kernel_spmd(nc, in_maps, core_ids=list(range(NCORES)))
    return assemble(res.results)


def kernel_traced(trace_cores=(0,), **inputs):
    """Like kernel() but returns (output, exec_time_ns, trace_path)."""
    nc, in_maps, assemble = _prepare(inputs)
    res = run_bass_kernel_spmd(nc, in_maps, core_ids=list(range(NCORES)),
                               trace=True, trace_cores=list(trace_cores))
    trace_path = None
    if res.instructions_and_trace is not None:
        trace_path = res.instructions_and_trace[1]
    return assemble(res.results), res.exec_time_ns, trace_path


# revision 19
# speedup vs baseline: 1.1847x; 1.0278x over previous
"""Trainium2 Bass kernel: DeformableValueAttention.

Full-input contract: kernel(**inputs) takes the unsharded inputs of
reference.setup_inputs() and returns the full [B, C, H, W] output.

Sharding: 8 cores = (batch b, query-half qh). Each core computes attention
for 512 queries x all 1024 keys of one batch and produces a disjoint
[C, 512] column-slice of the output -- no cross-core reduction.

Per-core algorithm (channels-on-partitions layouts):
  QT = (Wq/8) @ xq          [C, Nq]
  KT = Wk @ xkv             [C, N]
  V  = xkv^T @ Wv^T         [N, C]    (keys on partitions)
  Vd = G^T.T @ V            [N, C]    grid_sample as banded sparse matmul,
                                      (1+gamma*sal) folded into G
  per head pair t (2-head PE row-group packing, K=64):
    S^T = KT_t^T @ QT_t     [N, 2*Nq] scores, both heads side by side
    Pu  = exp(S^T)          bf16      (no max-subtraction: |scores| < ~6)
    O   = [Vd_h | 1]^T @ Pu [65, Nq]  per head; ones-column gives the
                                      softmax denominator in row 64
  normalize: DVE reciprocal of denom row + GPSIMD partition_broadcast +
             DVE multiply (no ACT -> no activation-table ping-pong);
             normalized heads land pairwise in [128, Nq] channel tiles
  out^T = Wo @ O            [C, Nq]   K=128 GEMM over the 4 pair tiles

Schedule notes (static in-order engine queues -> emission order is the
schedule):
  - Inputs arrive via one fat DMA per tensor, spread over the 4 engine
    queues in criticality order so the first exp fires ~6us earlier.
  - The 32 exp activations are the ACT pacer (~1.1us each); V, Vd and the
    first half of the O matmuls are interleaved into the S stream so the
    PE stays busy under the exp stream.
  - PSUM: two [128,1024] tags x 2 bufs = all 8 banks. O accumulates head A
    in bank cols 0:512 and head B in 512:1024 of one tile.
  - Output DMA is split per 128-row tile across all 4 queues.

Fidelity notes:
  - P_thermal adds a per-query constant pre-softmax; softmax is invariant.
  - Zero biases assumed (asserted); numpy fallback otherwise.
  - bf16 matmuls, fp32 PSUM; measured rel err ~5e-3 vs fp32 reference.
"""

import os
import sys

import numpy as np
import ml_dtypes

try:
    import concourse.bass as bass
except ImportError:  # pragma: no cover - path fallback for bare containers
    sys.path.insert(0, "/opt/trn_rl_repo")
    import concourse.bass as bass

import concourse.bacc as bacc
import concourse.tile as tile
from concourse import mybir
from concourse.bass_utils import run_bass_kernel_spmd

B, C, HH, WW = 4, 512, 32, 32
N = HH * WW          # 1024 spatial positions (= keys)
NQ = N // 2          # queries per core
NH, HD = 8, 64       # heads, head dim
P = 128
CT = C // P          # 4 channel partition-tiles
NKT = N // P         # 8 key tiles
NCORES = 8
BF16 = mybir.dt.bfloat16
FP32 = mybir.dt.float32
NP_BF16 = ml_dtypes.bfloat16
E = HD + 1           # head channels + ones column in vd tiles


# --------------------------------------------------------------------------
# host-side helpers
# --------------------------------------------------------------------------

def _gather_T(offsets_b, salf_b):
    """GT[k, n]: weight of source pixel k in grid-sampled output pixel n,
    with the per-source value modulation salf folded in. fp32 [N, N]."""
    ys = np.linspace(-1.0, 1.0, HH)
    xs = np.linspace(-1.0, 1.0, WW)
    gy, gx = np.meshgrid(ys, xs, indexing="ij")
    x = ((gx + offsets_b[0] / (WW / 2.0) + 1.0) * WW - 1.0) * 0.5
    y = ((gy + offsets_b[1] / (HH / 2.0) + 1.0) * HH - 1.0) * 0.5
    x = np.clip(x, 0.0, WW - 1.0)
    y = np.clip(y, 0.0, HH - 1.0)
    x0 = np.floor(x); y0 = np.floor(y)
    wx = x - x0; wy = y - y0
    x0i = x0.astype(np.int64); y0i = y0.astype(np.int64)
    x1i = np.minimum(x0i + 1, WW - 1); y1i = np.minimum(y0i + 1, HH - 1)
    GT = np.zeros((N, N), np.float32)
    n_idx = np.arange(N)
    for yi, xi, w in ((y0i, x0i, (1 - wx) * (1 - wy)),
                      (y0i, x1i, wx * (1 - wy)),
                      (y1i, x0i, (1 - wx) * wy),
                      (y1i, x1i, wx * wy)):
        np.add.at(GT, ((yi * WW + xi).reshape(-1), n_idx),
                  w.reshape(-1).astype(np.float32))
    GT *= salf_b[:, None]
    return GT


def _reference_numpy(q_feat, kv_feat, offsets, saliency_map, P_thermal,
                     Wq, bq, Wk, bk, Wv, bv, Wo, bo, lambda_p, gamma_val):
    """Plain numpy port of reference.py -- correctness fallback only."""
    qf = q_feat.reshape(B, C, N).transpose(0, 2, 1)
    kf = kv_feat.reshape(B, C, N).transpose(0, 2, 1)

    def heads(x, Wm, bm):
        return (x @ Wm.T + bm).reshape(B, N, NH, HD).transpose(0, 2, 1, 3)

    Q = heads(qf, Wq, bq)
    K = heads(kf, Wk, bk)
    V = heads(kf, Wv, bv)
    attn = np.einsum("bhqd,bhkd->bhqk", Q, K) * (HD ** -0.5)
    attn = attn + float(lambda_p) * P_thermal.reshape(B, 1, N, 1)
    attn = attn - attn.max(axis=-1, keepdims=True)
    w = np.exp(attn)
    w /= w.sum(axis=-1, keepdims=True)
    Vm = V * (1.0 + float(gamma_val) * saliency_map.reshape(B, 1, N, 1))
    Vsp = Vm.transpose(0, 2, 1, 3).reshape(B, N, C).transpose(0, 2, 1)
    Vd = np.empty_like(Vsp)
    for b in range(B):
        GT = _gather_T(offsets[b], np.ones(N, np.float32))
        Vd[b] = Vsp[b] @ GT
    Vdf = Vd.reshape(B, C, N).transpose(0, 2, 1).reshape(B, N, NH, HD).transpose(0, 2, 1, 3)
    out = np.einsum("bhqk,bhkd->bhqd", w, Vdf)
    out = out.transpose(0, 2, 1, 3).reshape(B, N, C)
    out = out @ Wo.T + bo
    return out.transpose(0, 2, 1).reshape(B, C, HH, WW).astype(np.float32)


# --------------------------------------------------------------------------
# device program
# --------------------------------------------------------------------------

def _build_program(chunks):
    """chunks: ordered list of (m, k) gather-tile pairs; same for all cores."""
    nch = len(chunks)
    chunks_for_m = {m: [] for m in range(NKT)}
    for idx, (m, k) in enumerate(chunks):
        chunks_for_m[m].append((idx, k))

    nc = bacc.Bacc(None, target_bir_lowering=False, debug=False)
    xq_d = nc.declare_dram_parameter("xq", [C, NQ], BF16, isOutput=False)
    xkv_d = nc.declare_dram_parameter("xkv", [C, N], BF16, isOutput=False)
    wq_d = nc.declare_dram_parameter("wqT", [C, C], BF16, isOutput=False)
    wk_d = nc.declare_dram_parameter("wkT", [C, C], BF16, isOutput=False)
    wv_d = nc.declare_dram_parameter("wvT", [C, C], BF16, isOutput=False)
    wo_d = nc.declare_dram_parameter("woT", [C, C], BF16, isOutput=False)
    gt_d = nc.declare_dram_parameter("gt", [nch, P, P], BF16, isOutput=False)
    out_d = nc.declare_dram_parameter("outT", [C, NQ], FP32, isOutput=True)

    with tile.TileContext(nc) as tc:
        with tc.tile_pool(name="const", bufs=1) as const, \
             tc.tile_pool(name="work", bufs=1) as work, \
             tc.tile_pool(name="pu_pool", bufs=1) as pu_pool, \
             tc.tile_pool(name="sm", bufs=4) as sm, \
             tc.tile_pool(name="psp", bufs=2, space="PSUM") as psp:

            # ---- fat input DMAs: one transfer per tensor, spread over the
            # four engine queues in consumption order.
            def fat_load(engine, dram, nm, width, row_lo=0, rows=C):
                """Load dram[row_lo:row_lo+rows, :width] as [128, rows//128 * width]."""
                kt_ = rows // P
                tl = const.tile([P, kt_ * width], BF16, name=nm, tag=nm)
                engine.dma_start(
                    out=tl[:].rearrange("p (k n) -> p k n", n=width),
                    in_=dram[row_lo:row_lo + rows, :].rearrange(
                        "(k p) n -> p k n", p=P))
                return tl

            # Only sync (SP), scalar (ACT) and gpsimd queues can issue DMAs.
            # criticality order per queue (~95GB/s each):
            #   gpsimd: xkv[0:128], xkv[128:256], wk
            #   sync:   wq, xkv[384:512], wv, wo
            #   scalar: xq, xkv[256:384], gt
            wk_w = fat_load(nc.gpsimd, wk_d, "wkw", C)
            wq_w = fat_load(nc.sync, wq_d, "wqw", C)
            xq_w = fat_load(nc.scalar, xq_d, "xqw", NQ)
            xkv_t = [None] * CT
            xkv_t[0] = fat_load(nc.gpsimd, xkv_d, "xkv0", N, row_lo=0, rows=P)
            xkv_t[3] = fat_load(nc.sync, xkv_d, "xkv3", N, row_lo=3 * P, rows=P)
            xkv_t[2] = fat_load(nc.scalar, xkv_d, "xkv2", N, row_lo=2 * P, rows=P)
            xkv_t[1] = fat_load(nc.gpsimd, xkv_d, "xkv1", N, row_lo=P, rows=P)
            wv_w = fat_load(nc.sync, wv_d, "wvw", C)
            gt_w = const.tile([P, nch * P], BF16, name="gtw", tag="gtw")
            nc.scalar.dma_start(
                out=gt_w[:].rearrange("p (c j) -> p c j", j=P),
                in_=gt_d[:].rearrange("c p j -> p c j"))
            wo_w = fat_load(nc.sync, wo_d, "wow", C)

            def wq_s(k, t):
                return wq_w[:, k * C + t * P: k * C + (t + 1) * P]

            def wk_s(k, t):
                return wk_w[:, k * C + t * P: k * C + (t + 1) * P]

            def xq_s(k):
                return xq_w[:, k * NQ:(k + 1) * NQ]

            def xkv_s(k, lo, width):
                return xkv_t[k][:, lo: lo + width]

            def wv_s(k):
                return wv_w[:, k * C:(k + 1) * C]

            # ---- SBUF result tiles ---------------------------------------
            # qt pairs: qt_sb[a][:, (t%2)*NQ:] = head-pair t   (t = 2a, 2a+1)
            qt_sb = [work.tile([P, 2 * NQ], BF16, name=f"qt{a}", tag=f"qt{a}")
                     for a in range(2)]
            kt_sb = [work.tile([P, N], BF16, name=f"kt{t}", tag=f"kt{t}")
                     for t in range(CT)]
            # v pairs: v_sb[j][:, (m%2)*C:] = keys tile m      (m = 2j, 2j+1)
            v_sb = [work.tile([P, 2 * C], BF16, name=f"v{j}", tag=f"v{j}")
                    for j in range(4)]
            # vd pairs: per m a [P, NH*E] block; cols h*E..h*E+63 head h,
            # col h*E+64 = 1.0 (softmax denominator trick)
            vd_sb = [work.tile([P, 2 * NH * E], BF16, name=f"vd{j}",
                               tag=f"vd{j}") for j in range(4)]
            o_pair = [work.tile([P, NQ], BF16, name=f"op{p}", tag=f"op{p}")
                      for p in range(CT)]
            pu_tiles = {}
            ps_o = [None] * CT

            def vd_l(m, h):
                j, r = m // 2, m % 2
                return vd_sb[j][:, r * NH * E + h * E: r * NH * E + (h + 1) * E]

            # ones columns of the vd tiles (written once, up front)
            for j in range(4):
                v3 = vd_sb[j][:].rearrange("p (rh e) -> p rh e", e=E)
                nc.vector.memset(v3[:, :, HD:E], 1.0)

            # ---- emission helpers ----------------------------------------
            def emit_qt_half(a, t, ps):
                for k in range(CT):
                    nc.tensor.matmul(ps[:, (t % 2) * NQ:(t % 2 + 1) * NQ],
                                     lhsT=wq_s(k, t), rhs=xq_s(k),
                                     start=(k == 0), stop=(k == CT - 1))

            def evac_qt_half(a, t, ps):
                nc.vector.tensor_copy(
                    qt_sb[a][:, (t % 2) * NQ:(t % 2 + 1) * NQ],
                    ps[:, (t % 2) * NQ:(t % 2 + 1) * NQ])

            def emit_kt_half(t, half, ps):
                # k-order matches xkv tile DMA arrival
                for i, k in enumerate((2, 3, 0, 1)):
                    nc.tensor.matmul(ps[:, half * NQ:(half + 1) * NQ],
                                     lhsT=wk_s(k, t),
                                     rhs=xkv_s(k, half * NQ, NQ),
                                     start=(i == 0), stop=(i == CT - 1))

            def evac_kt_half(t, half, ps):
                nc.vector.tensor_copy(
                    kt_sb[t][:, half * NQ:(half + 1) * NQ],
                    ps[:, half * NQ:(half + 1) * NQ])

            def emit_s_chunk(t, m):
                ps_s = psp.tile([P, 2 * NQ], FP32, name=f"pss{t}_{m}",
                                tag="ps_s", bufs=2)
                kt, qt = kt_sb[t], qt_sb[t // 2]
                qo = (t % 2) * NQ
                nc.tensor.matmul(ps_s[:, 0:NQ],
                                 lhsT=kt[0:HD, m * P:(m + 1) * P],
                                 rhs=qt[0:HD, qo:qo + NQ],
                                 start=True, stop=True)
                nc.tensor.matmul(ps_s[:, NQ:2 * NQ],
                                 lhsT=kt[HD:P, m * P:(m + 1) * P],
                                 rhs=qt[HD:P, qo:qo + NQ],
                                 start=True, stop=True)
                pu = pu_pool.tile([P, 2 * NQ], BF16, name=f"pu{t}_{m}",
                                  tag=f"pu{t}_{m}")
                nc.scalar.activation(out=pu[:], in_=ps_s[:],
                                     func=mybir.ActivationFunctionType.Exp)
                pu_tiles[(t, m)] = pu

            def emit_v_pair(j, ps):
                for m in (2 * j, 2 * j + 1):
                    for k in range(CT):
                        nc.tensor.matmul(ps[:, (m % 2) * C:(m % 2 + 1) * C],
                                         lhsT=xkv_s(k, m * P, P),
                                         rhs=wv_s(k),
                                         start=(k == 0), stop=(k == CT - 1))
                nc.vector.tensor_copy(v_sb[j][:], ps[:])

            def emit_vd_pair(j, ps):
                for m in (2 * j, 2 * j + 1):
                    lst = chunks_for_m[m]
                    for i, (idx, k) in enumerate(lst):
                        nc.tensor.matmul(
                            ps[:, (m % 2) * C:(m % 2 + 1) * C],
                            lhsT=gt_w[:, idx * P:(idx + 1) * P],
                            rhs=v_sb[k // 2][:, (k % 2) * C:(k % 2 + 1) * C],
                            start=(i == 0), stop=(i == len(lst) - 1))
                # strided evacuation into the [.., h, 0:64] sub-columns
                dst = vd_sb[j][:].rearrange("p (r h e) -> p r h e", e=E, h=NH)
                src = ps[:].rearrange("p (r h e) -> p r h e", e=HD, h=NH)
                nc.vector.tensor_copy(dst[:, :, :, 0:HD], src[:])

            def emit_o_chunk(hp, m, tag="acc"):
                # head pairs 2,3 accumulate in the ps_s slots (free once the
                # exp stream drains) so they never wait on pair 0/1 norms
                if ps_o[hp] is None:
                    ps_o[hp] = psp.tile([P, 2 * NQ], FP32, name=f"pso{hp}",
                                        tag=tag, bufs=2)
                ps = ps_o[hp]
                pu = pu_tiles[(hp, m)]
                nc.tensor.matmul(ps[0:E, 0:NQ],
                                 lhsT=vd_l(m, 2 * hp), rhs=pu[:, 0:NQ],
                                 start=(m == 0), stop=(m == NKT - 1))
                nc.tensor.matmul(ps[0:E, NQ:2 * NQ],
                                 lhsT=vd_l(m, 2 * hp + 1), rhs=pu[:, NQ:2 * NQ],
                                 start=(m == 0), stop=(m == NKT - 1))

            def emit_norm(hp):
                # heads 2hp (cols 0:NQ) and 2hp+1 (cols NQ:2NQ) of ps_o[hp]
                ps = ps_o[hp]
                for hh in range(2):
                    lo = hh * NQ
                    den = sm.tile([1, NQ], FP32, name=f"den{hp}_{hh}",
                                  tag=f"den{hp}_{hh}", bufs=1)
                    nc.vector.tensor_copy(den[:], ps[HD:E, lo:lo + NQ])
                    bc = sm.tile([HD, NQ], FP32, name=f"bc{hp}_{hh}",
                                 tag="bc", bufs=4)
                    nc.gpsimd.partition_broadcast(bc[:], den[:])
                    rec = sm.tile([HD, NQ], FP32, name=f"rec{hp}_{hh}",
                                  tag="rec", bufs=4)
                    # ~51 ULP approx; denominators are >= ~1 so the undefined
                    # edge cases (0/denorm/inf) cannot occur. The exact
                    # InstReciprocal measured 3.3us per call and serialized
                    # the whole output phase.
                    nc.vector.reciprocal_approx_fast(rec[:], bc[:])
                    nc.vector.tensor_mul(
                        o_pair[hp][hh * HD:(hh + 1) * HD, :],
                        ps[0:HD, lo:lo + NQ], rec[:])

            # ---- emission schedule ---------------------------------------
            # warmup: KT(0) first (longest DMA dependency chain), QT pairs
            # as filler, then the exp-paced S stream with V/Vd/O interleave.
            # PSUM tiles are allocated strictly in use order so the acc-tag
            # round-robin (bufs=2) matches the schedule.
            # global S-chunk order: (t, m) lexicographic; each emission site
            # pulls the next chunk so the exp stream never reorders.
            s_seq = [(t, m) for t in range(CT) for m in range(NKT)]
            s_pos = [0]

            def next_s(n=1):
                for _ in range(n):
                    t, m = s_seq[s_pos[0]]
                    s_pos[0] += 1
                    emit_s_chunk(t, m)

            # warmup: QT half t=0 fills the xkv DMA wait, then KT(0) half A
            # unlocks S(0,0..3) as early as possible.
            psq0 = psp.tile([P, 2 * NQ], FP32, name="psq0", tag="acc", bufs=2)
            emit_qt_half(0, 0, psq0)
            psk0 = psp.tile([P, 2 * NQ], FP32, name="psk0", tag="acc", bufs=2)
            emit_kt_half(0, 0, psk0)
            evac_qt_half(0, 0, psq0)
            evac_kt_half(0, 0, psk0)
            next_s(2)                     # S(0,0), S(0,1)
            emit_kt_half(0, 1, psk0)
            evac_kt_half(0, 1, psk0)
            next_s(2)                     # S(0,2), S(0,3)
            emit_qt_half(0, 1, psq0)
            evac_qt_half(0, 1, psq0)
            # KT(1..3) + QT pair 1 interleaved into the exp-paced stream
            for t in (1, 2, 3):
                pskt = psp.tile([P, 2 * NQ], FP32, name=f"psk{t}", tag="acc",
                                bufs=2)
                for half in range(2):
                    emit_kt_half(t, half, pskt)
                    evac_kt_half(t, half, pskt)
                    next_s(1)
                if t == 1:
                    psq1 = psp.tile([P, 2 * NQ], FP32, name="psq1", tag="acc",
                                    bufs=2)
                    for tt in (2, 3):
                        emit_qt_half(1, tt, psq1)
                        evac_qt_half(1, tt, psq1)
            # V pairs: one per two exp slots
            for j in range(4):
                ps = psp.tile([P, 2 * C], FP32, name=f"psv{j}", tag="acc",
                              bufs=2)
                next_s(1)
                emit_v_pair(j, ps)
                next_s(1)
            # Vd pairs
            for j in range(4):
                ps = psp.tile([P, 2 * C], FP32, name=f"psvd{j}", tag="acc",
                              bufs=2)
                next_s(1)
                emit_vd_pair(j, ps)
                next_s(1)
            # remaining S slots: interleave O chunks for head pairs 0 and 1
            o01 = [(0, m) for m in range(NKT)] + [(1, m) for m in range(NKT)]
            oi = 0
            while s_pos[0] < len(s_seq):
                next_s(1)
                for _ in range(2):
                    if oi < len(o01):
                        hp, m = o01[oi]
                        emit_o_chunk(hp, m)
                        oi += 1
            while oi < len(o01):
                hp, m = o01[oi]
                emit_o_chunk(hp, m)
                oi += 1
            emit_norm(0)
            emit_norm(1)
            for m in range(NKT):
                emit_o_chunk(2, m, tag="ps_s")
            emit_norm(2)
            for m in range(NKT):
                emit_o_chunk(3, m, tag="ps_s")
            emit_norm(3)

            # ---- out^T = Wo @ O : [C, NQ] fp32 ---------------------------
            # K=128 GEMM over the four normalized pair tiles; per 128-row
            # output tile: accumulate, evacuate, DMA in two row-halves
            # spread over the three DMA-capable queues.
            # p-outer emission: the 12 matmuls over pairs 0-2 run while the
            # last pair is still normalizing; only the four p=3 matmuls sit
            # behind the final norm.
            out_q = [(nc.sync, nc.scalar), (nc.gpsimd, nc.sync),
                     (nc.scalar, nc.gpsimd), (nc.sync, nc.scalar)]
            ps_w = [psp.tile([P, 2 * NQ], FP32, name=f"psw{th}", tag="acc",
                             bufs=2) for th in range(2)]
            for p in range(CT):
                for t in range(CT):
                    ps = ps_w[t // 2]
                    nc.tensor.matmul(
                        ps[:, (t % 2) * NQ:(t % 2 + 1) * NQ],
                        lhsT=wo_w[:, p * C + t * P: p * C + (t + 1) * P],
                        rhs=o_pair[p][:],
                        start=(p == 0), stop=(p == CT - 1))
                    if p == CT - 1:
                        obt = sm.tile([P, NQ], FP32, name=f"ob{t}", tag="ob",
                                      bufs=2)
                        nc.vector.tensor_copy(
                            obt[:], ps[:, (t % 2) * NQ:(t % 2 + 1) * NQ])
                        qa, qb = out_q[t]
                        qa.dma_start(out=out_d[t * P:t * P + 64, :],
                                     in_=obt[0:64, :])
                        qb.dma_start(out=out_d[t * P + 64:(t + 1) * P, :],
                                     in_=obt[64:P, :])

    nc.compile()
    return nc


# --------------------------------------------------------------------------
# public entry points
# --------------------------------------------------------------------------

def _prepare(inputs):
    q = np.ascontiguousarray(inputs["q_feat"], np.float32).reshape(B, C, N)
    kv = np.ascontiguousarray(inputs["kv_feat"], np.float32).reshape(B, C, N)
    offsets = np.asarray(inputs["offsets"], np.float32)
    sal = np.asarray(inputs["saliency_map"], np.float32).reshape(B, N)
    gamma = float(np.asarray(inputs["gamma_val"]))

    GTs = [_gather_T(offsets[b], 1.0 + gamma * sal[b]) for b in range(B)]

    # union band-sparsity pattern of the gather matmul across batches, so the
    # SPMD program is identical on every core
    chunks = []
    for m in range(NKT):
        for k in range(NKT):
            if any(GTs[b][k * P:(k + 1) * P, m * P:(m + 1) * P].any()
                   for b in range(B)):
                chunks.append((m, k))

    wqT = np.ascontiguousarray((np.asarray(inputs["Wq"], np.float32).T
                                * (HD ** -0.5)).astype(NP_BF16))
    wkT = np.ascontiguousarray(np.asarray(inputs["Wk"], np.float32).T.astype(NP_BF16))
    wvT = np.ascontiguousarray(np.asarray(inputs["Wv"], np.float32).T.astype(NP_BF16))
    woT = np.ascontiguousarray(np.asarray(inputs["Wo"], np.float32).T.astype(NP_BF16))

    in_maps = []
    for core in range(NCORES):
        b, qh = core // 2, core % 2
        gt_stack = np.stack([
            np.ascontiguousarray(
                GTs[b][k * P:(k + 1) * P, m * P:(m + 1) * P]).astype(NP_BF16)
            for (m, k) in chunks])
        in_maps.append({
            "xq": np.ascontiguousarray(
                q[b][:, qh * NQ:(qh + 1) * NQ]).astype(NP_BF16),
            "xkv": np.ascontiguousarray(kv[b]).astype(NP_BF16),
            "wqT": wqT, "wkT": wkT, "wvT": wvT, "woT": woT,
            "gt": gt_stack,
        })

    def assemble(results):
        out = np.empty((B, C, N), np.float32)
        for core in range(NCORES):
            b, qh = core // 2, core % 2
            out[b][:, qh * NQ:(qh + 1) * NQ] = results[core]["outT"]
        return out.reshape(B, C, HH, WW)

    nc = _build_program(chunks)
    return nc, in_maps, assemble


def _needs_fallback(inputs):
    try:
        if tuple(np.shape(inputs["q_feat"])) != (B, C, HH, WW):
            return True
        for bias in ("bq", "bk", "bv", "bo"):
            if np.any(np.asarray(inputs[bias], np.float32) != 0.0):
                return True
    except Exception:
        return True
    return False


def kernel(**inputs) -> np.ndarray:
    if _needs_fallback(inputs):
        return _reference_numpy(**{k: np.asarray(v, np.float32)
                                   for k, v in inputs.items()})
    nc, in_maps, assemble = _prepare(inputs)
    res = run_bass_kernel_spmd(nc, in_maps, core_ids=list(range(NCORES)))
    return assemble(res.results)


def kernel_traced(trace_cores=(0,), **inputs):
    """Like kernel() but returns (output, exec_time_ns, trace_path)."""
    nc, in_maps, assemble = _prepare(inputs)
    res = run_bass_kernel_spmd(nc, in_maps, core_ids=list(range(NCORES)),
                               trace=True, trace_cores=list(trace_cores))
    trace_path = None
    if res.instructions_and_trace is not None:
        trace_path = res.instructions_and_trace[1]
    return assemble(res.results), res.exec_time_ns, trace_path
